# revision 15
# baseline (speedup 1.0000x reference)
"""Trainium2 Bass kernel for nn_BinaryCorrelationMatcher.

Math: reference corr(dy,dx) = box21(p1*s + (1-p1)*(1-s)) with s = shift(p2).
Since box21(agreement) = box21(1-p1) + box21((2*p1-1)*s) and the first term is
shift-independent, the argmax over the 361 shifts only needs
m(dy,dx) = box21(q1 * shift(p2)), q1 = 2*p1-1.  max_score itself is never an
output (only flow + confidence), so the offset term is dropped entirely.

Per core (8 cores = 4 batches x 2 W-halves): all 480 rows, 320 output cols.
Pipeline per (dy, dx): prod = q1*shifted_p2 (DVE) -> x-cumsum (DVE scan) ->
column box + x-box-diff fused into +/-banded fp32 matmuls (PE, PSUM accum) ->
per-dy prefix-max chain + count-below-max = exact first-wins argmax (DVE/GPSIMD).
"""

import numpy as np
import sys

for _p in ("/opt/trn_rl_repo", "/root/.axon_site/_ro/trn_rl_repo"):
    if _p not in sys.path:
        sys.path.append(_p)

from contextlib import ExitStack

import concourse.bass as bass
import concourse.bacc as bacc
import concourse.tile as tile
from concourse import mybir
from concourse.bass_utils import run_bass_kernel_spmd

F32 = mybir.dt.float32
ALU = mybir.AluOpType

H, W = 480, 640
JW = 320          # output cols per core
XW = 342          # cum slab width (x'=0 pad col, x'=1..341 real, diff = cum[j+21]-cum[j])
P2W = 360         # p2 slab cols
YROWS = 500       # prod rows [-10, 490)
P2ROWS = 518      # p2 rows [-19, 499)

# y-tiles of the prod/cum slab (slab row ranges)
YT = [(0, 128), (128, 256), (256, 384), (384, 500)]
# out tiles: (i0, i1, pieces); piece = (yk, r0, r1) meaning cum tile yk rows [r0,r1)
OT = [
    (0, 108, [(0, 0, 128)]),
    (108, 236, [(0, 96, 128), (1, 0, 128)]),
    (236, 364, [(1, 96, 128), (2, 0, 128)]),
    (364, 480, [(2, 96, 128), (3, 0, 116)]),
]
CHUNKS = [(c, min(26, JW - c)) for c in range(0, JW, 26)]  # 13 chunks
NDY = 19


def _band_index():
    """k index for weight tile of (otile t, piece p, sign s): 2*(cum pieces so far)+ (0 plus,1 minus)"""
    idx = {}
    k = 0
    for t, (i0, i1, pieces) in enumerate(OT):
        for p in range(len(pieces)):
            for si, s in enumerate((+1, -1)):
                idx[(t, p, si)] = k
                k += 1
    return idx, k


BIDX, NBANDS = _band_index()


def make_bands():
    bands = np.zeros((128, NBANDS, 128), np.float32)
    for t, (i0, i1, pieces) in enumerate(OT):
        T = i1 - i0
        for p, (yk, r0, r1) in enumerate(pieces):
            rows = r1 - r0
            rr = np.arange(rows)[:, None]
            ii = np.arange(T)[None, :]
            gy = 128 * yk + r0 + rr - 10   # global prod row
            gi = i0 + ii                   # global out row
            w = (np.abs(gy - gi) <= 10).astype(np.float32)
            for si, s in enumerate((+1, -1)):
                # stored at partitions [r0:r1) so lhsT/rhs base partitions match
                bands[r0:r1, BIDX[(t, p, si)], :T] = s * w
    return bands


def build_program():
    nc = bacc.Bacc("TRN2", target_bir_lowering=False, debug=False)
    p1s_d = nc.declare_dram_parameter("p1s", [YROWS, XW], F32, isOutput=False)
    q1s_d = nc.declare_dram_parameter("q1s", [YROWS, XW], F32, isOutput=False)
    p2s_d = nc.declare_dram_parameter("p2s", [P2ROWS, P2W], F32, isOutput=False)
    bands_d = nc.declare_dram_parameter("bands", [128, NBANDS, 128], F32, isOutput=False)
    dyv_d = nc.declare_dram_parameter("dyv", [128, NDY], F32, isOutput=False)
    fx_d = nc.declare_dram_parameter("fx", [H, JW], F32, isOutput=True)
    fy_d = nc.declare_dram_parameter("fy", [H, JW], F32, isOutput=True)
    cf_d = nc.declare_dram_parameter("cf", [H, JW], F32, isOutput=True)

    with tile.TileContext(nc) as tc, ExitStack() as ctx:
        const = ctx.enter_context(tc.tile_pool(name="const", bufs=1))
        planes = ctx.enter_context(tc.tile_pool(name="planes", bufs=1))
        p2pool = ctx.enter_context(tc.tile_pool(name="p2", bufs=2))
        prodpool = ctx.enter_context(tc.tile_pool(name="prod", bufs=1))
        cumpool = ctx.enter_context(tc.tile_pool(name="cum", bufs=2))
        mpool = ctx.enter_context(tc.tile_pool(name="mslab", bufs=1))
        psm = ctx.enter_context(tc.tile_pool(name="psm", bufs=5, space="PSUM"))
        psc = ctx.enter_context(tc.tile_pool(name="psc", bufs=2, space="PSUM"))

        p1t = const.tile([128, 4, XW], F32)
        q1t = const.tile([128, 4, XW], F32)
        bands_t = const.tile([128, NBANDS, 128], F32)
        dyv_t = const.tile([128, NDY], F32)
        zeros_t = const.tile([128, XW], F32)
        # best / fx / fy / cf planes packed: [128, plane, otile, JW]
        pl_t = planes.tile([128, 4, 4, JW], F32)
        BEST, FX, FY, CF = 0, 1, 2, 3
        tmp_t = const.tile([128, 3, JW], F32)  # bm, cnt, (spare)
        BM, CNT, GT = 0, 1, 2
        gt8_t = const.tile([128, JW], mybir.dt.uint8)

        for k, (r0, r1) in enumerate(YT):
            nc.sync.dma_start(p1t[0 : r1 - r0, k, :], p1s_d[r0:r1, :])
        nc.sync.dma_start(bands_t[:, :, :], bands_d[:, :, :])
        nc.sync.dma_start(dyv_t[:, :], dyv_d[:, :])
        nc.vector.memset(zeros_t[:, :], 0.0)
        nc.vector.memset(pl_t[:, BEST, :, :], -1.0e30)
        nc.vector.memset(pl_t[:, FX, :, :], 0.0)
        nc.vector.memset(pl_t[:, FY, :, :], 0.0)
        for k, (r0, r1) in enumerate(YT):
            nc.sync.dma_start(q1t[0 : r1 - r0, k, :], q1s_d[r0:r1, :])

        # ---- confidence: box21(p1)/441 clipped to [0,1] ----
        ccum = cumpool.tile([128, NDY, XW], F32, tag="cum")
        for k, (r0, r1) in enumerate(YT):
            rows = r1 - r0
            nc.vector.tensor_tensor_scan(
                ccum[0:rows, k, :], p1t[0:rows, k, :], zeros_t[0:rows, :],
                0.0, ALU.add, ALU.add,
            )
        for t, (i0, i1, pieces) in enumerate(OT):
            T = i1 - i0
            pc = psc.tile([128, JW], F32)
            nmm = 2 * len(pieces)
            mi = 0
            for p, (yk, r0, r1) in enumerate(pieces):
                tp = (r0, 0) if r0 else None
                for si, off in ((0, 21), (1, 0)):
                    nc.tensor.matmul(
                        pc[0:T, :],
                        bands_t[r0:r1, BIDX[(t, p, si)], 0:T],
                        ccum[r0:r1, yk, off : off + JW],
                        start=(mi == 0),
                        stop=(mi == nmm - 1),
                        tile_position=tp,
                    )
                    mi += 1
            nc.vector.tensor_scalar(
                pl_t[0:T, CF, t, :], pc[0:T, :], 1.0 / 441.0, 1.0, ALU.mult, ALU.min
            )

        # ---- main loop over dy ----
        for dyi in range(NDY):
            dy = dyi - 9
            cums = {}

            def make_cum(k):
                r0, r1 = YT[k]
                rows = r1 - r0
                p2t = p2pool.tile([128, P2W], F32)
                nc.sync.dma_start(
                    p2t[0:rows, :], p2s_d[r0 + dy + 9 : r0 + dy + 9 + rows, :]
                )
                prod = prodpool.tile([128, NDY, XW], F32)
                cum = cumpool.tile([128, NDY, XW], F32, tag="cum")
                for dxi in range(NDY):
                    nc.vector.tensor_tensor(
                        prod[0:rows, dxi, :],
                        q1t[0:rows, k, :],
                        p2t[0:rows, dxi : dxi + XW],
                        ALU.mult,
                    )
                    nc.vector.tensor_tensor_scan(
                        cum[0:rows, dxi, :], prod[0:rows, dxi, :],
                        zeros_t[0:rows, :], 0.0, ALU.add, ALU.add,
                    )
                cums[k] = cum

            for t, (i0, i1, pieces) in enumerate(OT):
                make_cum(t)  # otile t needs cum tiles up to Y_t
                T = i1 - i0
                mslab = mpool.tile([128, JW, NDY], F32)
                for (c0, cn) in CHUNKS:
                    ps = psm.tile([128, 26, NDY], F32)
                    nmm = 2 * len(pieces)
                    mi = 0
                    for p, (yk, r0, r1) in enumerate(pieces):
                        tp = (r0, 0) if r0 else None
                        for si, off in ((0, 21), (1, 0)):
                            rhs = cums[yk][r0:r1, :, off + c0 : off + c0 + cn].rearrange(
                                "p dx x -> p x dx"
                            )
                            nc.tensor.matmul(
                                ps[0:T, 0:cn, :],
                                bands_t[r0:r1, BIDX[(t, p, si)], 0:T],
                                rhs,
                                start=(mi == 0),
                                stop=(mi == nmm - 1),
                                tile_position=tp,
                            )
                            mi += 1
                    nc.scalar.copy(mslab[0:T, c0 : c0 + cn, :], ps[0:T, 0:cn, :])

                # prefix-max chain along dx (in-place)
                for dxi in range(1, NDY):
                    nc.vector.tensor_tensor(
                        mslab[0:T, :, dxi],
                        mslab[0:T, :, dxi - 1],
                        mslab[0:T, :, dxi],
                        ALU.max,
                    )
                # block max
                nc.scalar.copy(tmp_t[0:T, BM, :], mslab[0:T, :, 18])
                # mask = pm < bm (in place), then cnt = sum(mask) = first-argmax idx
                bmb = tmp_t[0:T, BM, :].unsqueeze(2).broadcast_to([T, JW, NDY])
                nc.vector.tensor_tensor(
                    mslab[0:T, :, :], mslab[0:T, :, :], bmb, ALU.is_lt
                )
                nc.vector.tensor_reduce(
                    tmp_t[0:T, CNT, :], mslab[0:T, :, :], mybir.AxisListType.X, ALU.add
                )
                # cross-dy update
                nc.vector.tensor_tensor(
                    gt8_t[0:T, :], tmp_t[0:T, BM, :], pl_t[0:T, BEST, t, :], ALU.is_gt
                )
                nc.vector.tensor_tensor(
                    pl_t[0:T, BEST, t, :], pl_t[0:T, BEST, t, :], tmp_t[0:T, BM, :], ALU.max
                )
                nc.vector.tensor_scalar(
                    tmp_t[0:T, CNT, :], tmp_t[0:T, CNT, :], -9.0, None, ALU.add, ALU.bypass
                )
                nc.vector.copy_predicated(
                    pl_t[0:T, FX, t, :], gt8_t[0:T, :], tmp_t[0:T, CNT, :]
                )
                dyb = dyv_t[0:T, dyi : dyi + 1].broadcast_to([T, JW])
                nc.vector.copy_predicated(pl_t[0:T, FY, t, :], gt8_t[0:T, :], dyb)

        for t, (i0, i1, _) in enumerate(OT):
            T = i1 - i0
            nc.sync.dma_start(fx_d[i0:i1, :], pl_t[0:T, FX, t, :])
            nc.sync.dma_start(fy_d[i0:i1, :], pl_t[0:T, FY, t, :])
            nc.sync.dma_start(cf_d[i0:i1, :], pl_t[0:T, CF, t, :])

    nc.compile()
    return nc


_NC = None


def get_program():
    global _NC
    if _NC is None:
        _NC = build_program()
    return _NC


def make_core_inputs(p1b, p2b, j0, bands, dyv):
    p1p = np.pad(p1b, ((10, 10), (11, 11)))
    p1s = p1p[:, j0 : j0 + XW].copy()
    q1p = np.pad((2.0 * p1b - 1.0).astype(np.float32), ((10, 10), (11, 11)))
    q1s = q1p[:, j0 : j0 + XW].copy()
    q1s[:, 0] = 0.0
    p2p = np.pad(p2b, ((19, 19), (20, 20)))
    p2s = p2p[:, j0 : j0 + P2W].copy()
    return {
        "p1s": np.ascontiguousarray(p1s, np.float32),
        "q1s": np.ascontiguousarray(q1s, np.float32),
        "p2s": np.ascontiguousarray(p2s, np.float32),
        "bands": bands,
        "dyv": dyv,
    }


def kernel(p1, p2):
    p1 = np.asarray(p1, np.float32)
    p2 = np.asarray(p2, np.float32)
    B = p1.shape[0]
    nc = get_program()
    bands = make_bands()
    dyv = np.tile(np.arange(-9, 10, dtype=np.float32), (128, 1))

    in_maps = []
    cores = []
    for b in range(B):
        for wh in range(2):
            j0 = wh * JW
            in_maps.append(make_core_inputs(p1[b, 0], p2[b, 0], j0, bands, dyv))
            cores.append((b, wh))

    res = run_bass_kernel_spmd(nc, in_maps, list(range(len(in_maps))))

    flow = np.zeros((B, 2, H, W), np.float32)
    conf = np.zeros((B, 1, H, W), np.float32)
    for i, (b, wh) in enumerate(cores):
        j0 = wh * JW
        r = res.results[i]
        flow[b, 0, :, j0 : j0 + JW] = r["fx"]
        flow[b, 1, :, j0 : j0 + JW] = r["fy"]
        conf[b, 0, :, j0 : j0 + JW] = r["cf"]
    return flow, conf


# revision 16
# speedup vs baseline: 16.2075x; 16.2075x over previous
"""Trainium2 Bass kernel for nn_BinaryCorrelationMatcher.

Math: reference corr(dy,dx) = box21(p1*s + (1-p1)*(1-s)) with s = shift(p2).
Since box21(agreement) = box21(1-p1) + box21((2*p1-1)*s) and the first term is
shift-independent, the argmax over the 361 shifts only needs
m(dy,dx) = box21(q1 * shift(p2)), q1 = 2*p1-1.  max_score itself is never an
output (only flow + confidence), so the offset term is dropped entirely.

Per core (8 cores = 4 batches x 2 W-halves): all 480 rows, 320 output cols.
Pipeline per (dy, dx): prod = q1*shifted_p2 (DVE) -> x-cumsum (DVE scan) ->
column box + x-box-diff fused into +/-banded fp32 matmuls (PE, PSUM accum) ->
per-dy prefix-max chain + count-below-max = exact first-wins argmax (DVE/GPSIMD).
"""

import numpy as np
import sys

for _p in ("/opt/trn_rl_repo", "/root/.axon_site/_ro/trn_rl_repo"):
    if _p not in sys.path:
        sys.path.append(_p)

from contextlib import ExitStack

import concourse.bass as bass
import concourse.bacc as bacc
import concourse.tile as tile
from concourse import mybir
from concourse.bass_utils import run_bass_kernel_spmd

F32 = mybir.dt.float32
ALU = mybir.AluOpType

H, W = 480, 640
JW = 320          # output cols per core
XW = 342          # cum slab width (x'=0 pad col, x'=1..341 real, diff = cum[j+21]-cum[j])
P2W = 360         # p2 slab cols
YROWS = 500       # prod rows [-10, 490)
P2ROWS = 518      # p2 rows [-19, 499)

# y-tiles of the prod/cum slab (slab row ranges)
YT = [(0, 128), (128, 256), (256, 384), (384, 500)]
# out tiles: (i0, i1, pieces); piece = (yk, r0, r1) meaning cum tile yk rows [r0,r1)
OT = [
    (0, 108, [(0, 0, 128)]),
    (108, 236, [(0, 96, 128), (1, 0, 128)]),
    (236, 364, [(1, 96, 128), (2, 0, 128)]),
    (364, 480, [(2, 96, 128), (3, 0, 116)]),
]
CHUNKS = [(c, min(26, JW - c)) for c in range(0, JW, 26)]  # 13 chunks
MCHUNKS = [(c, min(26, XW - 1 - c)) for c in range(0, XW - 1, 26)]  # 14 chunks over x in [0,341)
NDY = 19


def _band_index():
    """k index for weight tile of (otile t, piece p, sign s): 2*(cum pieces so far)+ (0 plus,1 minus)"""
    idx = {}
    k = 0
    for t, (i0, i1, pieces) in enumerate(OT):
        for p in range(len(pieces)):
            for si, s in enumerate((+1, -1)):
                idx[(t, p, si)] = k
                k += 1
    return idx, k


BIDX, NBANDS = _band_index()


def make_bands():
    bands = np.zeros((128, NBANDS, 128), np.float32)
    for t, (i0, i1, pieces) in enumerate(OT):
        T = i1 - i0
        for p, (yk, r0, r1) in enumerate(pieces):
            rows = r1 - r0
            rr = np.arange(rows)[:, None]
            ii = np.arange(T)[None, :]
            gy = 128 * yk + r0 + rr - 10   # global prod row
            gi = i0 + ii                   # global out row
            w = (np.abs(gy - gi) <= 10).astype(np.float32)
            for si, s in enumerate((+1, -1)):
                # stored at partitions [r0:r1) so lhsT/rhs base partitions match
                bands[r0:r1, BIDX[(t, p, si)], :T] = s * w
    return bands


def build_program():
    nc = bacc.Bacc("TRN2", target_bir_lowering=False, debug=False)
    p1s_d = nc.declare_dram_parameter("p1s", [YROWS, XW], F32, isOutput=False)
    q1s_d = nc.declare_dram_parameter("q1s", [YROWS, XW], F32, isOutput=False)
    p2s_d = nc.declare_dram_parameter("p2s", [P2ROWS, P2W], F32, isOutput=False)
    bands_d = nc.declare_dram_parameter("bands", [128, NBANDS, 128], F32, isOutput=False)
    dyv_d = nc.declare_dram_parameter("dyv", [128, NDY], F32, isOutput=False)
    fx_d = nc.declare_dram_parameter("fx", [H, JW], F32, isOutput=True)
    fy_d = nc.declare_dram_parameter("fy", [H, JW], F32, isOutput=True)
    cf_d = nc.declare_dram_parameter("cf", [H, JW], F32, isOutput=True)

    with tile.TileContext(nc) as tc, ExitStack() as ctx:
        const = ctx.enter_context(tc.tile_pool(name="const", bufs=1))
        planes = ctx.enter_context(tc.tile_pool(name="planes", bufs=1))
        p2pool = ctx.enter_context(tc.tile_pool(name="p2", bufs=2))
        prodpool = ctx.enter_context(tc.tile_pool(name="prod", bufs=1))
        cumpool = ctx.enter_context(tc.tile_pool(name="cum", bufs=2))
        mprepool = ctx.enter_context(tc.tile_pool(name="mpre", bufs=1))
        mpool = ctx.enter_context(tc.tile_pool(name="mslab", bufs=1))
        psm = ctx.enter_context(tc.tile_pool(name="psm", bufs=5, space="PSUM"))
        psc = ctx.enter_context(tc.tile_pool(name="psc", bufs=2, space="PSUM"))

        p1t = const.tile([128, 4, XW], F32)
        q1t = const.tile([128, 4, XW], F32)
        bands_t = const.tile([128, NBANDS, 128], F32)
        dyv_t = const.tile([128, NDY], F32)
        zeros_t = const.tile([128, XW], F32)
        # best / fx / fy / cf planes packed: [128, plane, otile, JW]
        pl_t = planes.tile([128, 4, 4, JW], F32)
        BEST, FX, FY, CF = 0, 1, 2, 3
        tmp_t = const.tile([128, 3, JW], F32)  # bm, cnt, (spare)
        BM, CNT, GT = 0, 1, 2
        gt8_t = const.tile([128, JW], mybir.dt.uint8)

        for k, (r0, r1) in enumerate(YT):
            nc.sync.dma_start(p1t[0 : r1 - r0, k, :], p1s_d[r0:r1, :])
        nc.sync.dma_start(bands_t[:, :, :], bands_d[:, :, :])
        nc.sync.dma_start(dyv_t[:, :], dyv_d[:, :])
        nc.vector.memset(zeros_t[:, :], 0.0)
        nc.vector.memset(pl_t[:, BEST, :, :], -1.0e30)
        nc.vector.memset(pl_t[:, FX, :, :], 0.0)
        nc.vector.memset(pl_t[:, FY, :, :], 0.0)
        for k, (r0, r1) in enumerate(YT):
            nc.sync.dma_start(q1t[0 : r1 - r0, k, :], q1s_d[r0:r1, :])

        # ---- confidence: box21(p1)/441 clipped to [0,1] ----
        ccum = cumpool.tile([128, NDY, XW], F32, tag="cum")
        for k, (r0, r1) in enumerate(YT):
            rows = r1 - r0
            nc.vector.tensor_tensor_scan(
                ccum[0:rows, k, :], p1t[0:rows, k, :], zeros_t[0:rows, :],
                0.0, ALU.add, ALU.add,
            )
        for t, (i0, i1, pieces) in enumerate(OT):
            T = i1 - i0
            pc = psc.tile([128, JW], F32)
            nmm = 2 * len(pieces)
            mi = 0
            for p, (yk, r0, r1) in enumerate(pieces):
                tp = (r0, 0) if r0 else None
                for si, off in ((0, 21), (1, 0)):
                    nc.tensor.matmul(
                        pc[0:T, :],
                        bands_t[r0:r1, BIDX[(t, p, si)], 0:T],
                        ccum[r0:r1, yk, off : off + JW],
                        start=(mi == 0),
                        stop=(mi == nmm - 1),
                        tile_position=tp,
                    )
                    mi += 1
            nc.vector.tensor_scalar(
                pl_t[0:T, CF, t, :], pc[0:T, :], 1.0 / 441.0, 1.0, ALU.mult, ALU.min
            )

        # ---- main loop over dy ----
        for dyi in range(NDY):
            dy = dyi - 9
            cums = {}

            def make_cum(k):
                r0, r1 = YT[k]
                rows = r1 - r0
                p2t = p2pool.tile([128, P2W], F32)
                nc.sync.dma_start(
                    p2t[0:rows, :], p2s_d[r0 + dy + 9 : r0 + dy + 9 + rows, :]
                )
                prod = prodpool.tile([128, NDY, XW], F32)
                cum = cumpool.tile([128, NDY, XW], F32, tag="cum")
                for dxi in range(NDY):
                    nc.vector.tensor_tensor(
                        prod[0:rows, dxi, :],
                        q1t[0:rows, k, :],
                        p2t[0:rows, dxi : dxi + XW],
                        ALU.mult,
                    )
                    nc.vector.tensor_tensor_scan(
                        cum[0:rows, dxi, :], prod[0:rows, dxi, :],
                        zeros_t[0:rows, :], 0.0, ALU.add, ALU.add,
                    )
                cums[k] = cum

            for t, (i0, i1, pieces) in enumerate(OT):
                make_cum(t)  # otile t needs cum tiles up to Y_t
                T = i1 - i0
                mpre = mprepool.tile([128, XW - 1, NDY], F32)   # colbox(cum), x-major
                mslab = mpool.tile([128, NDY, JW], F32)          # diffed m, dx-major
                for (c0, cn) in MCHUNKS:
                    ps = psm.tile([128, 26, NDY], F32)
                    nmm = len(pieces)
                    for p, (yk, r0, r1) in enumerate(pieces):
                        tp = (r0, 0) if r0 else None
                        rhs = cums[yk][r0:r1, :, c0 : c0 + cn].rearrange(
                            "p dx x -> p x dx"
                        )
                        nc.tensor.matmul(
                            ps[0:T, 0:cn, :],
                            bands_t[r0:r1, BIDX[(t, p, 0)], 0:T],
                            rhs,
                            start=(p == 0),
                            stop=(p == nmm - 1),
                            tile_position=tp,
                        )
                    nc.scalar.copy(mpre[0:T, c0 : c0 + cn, :], ps[0:T, 0:cn, :])

                # m = colbox(cum)[x+21] - colbox(cum)[x]  (x-box diff), dx-major out
                nc.vector.tensor_tensor(
                    mslab[0:T, :, :],
                    mpre[0:T, 21 : 21 + JW, :].rearrange("p x dx -> p dx x"),
                    mpre[0:T, 0:JW, :].rearrange("p x dx -> p dx x"),
                    ALU.subtract,
                )
                # prefix-max chain along dx (in-place, contiguous rows)
                for dxi in range(1, NDY):
                    nc.vector.tensor_tensor(
                        mslab[0:T, dxi, :],
                        mslab[0:T, dxi - 1, :],
                        mslab[0:T, dxi, :],
                        ALU.max,
                    )
                # block max (save before mask overwrites the slab)
                nc.scalar.copy(tmp_t[0:T, BM, :], mslab[0:T, 18, :])
                # mask = pm < bm (in place), cnt = sum(mask) = first-argmax idx
                bmb = mslab[0:T, 18:19, :].broadcast_to([T, NDY, JW])
                nc.vector.tensor_tensor(
                    mslab[0:T, :, :], mslab[0:T, :, :], bmb, ALU.is_lt
                )
                nc.vector.tensor_tensor(
                    tmp_t[0:T, CNT, :], mslab[0:T, 0, :], mslab[0:T, 1, :], ALU.add
                )
                for dxi in range(2, NDY):
                    nc.vector.tensor_tensor(
                        tmp_t[0:T, CNT, :], tmp_t[0:T, CNT, :], mslab[0:T, dxi, :], ALU.add
                    )
                # cross-dy update
                nc.vector.tensor_tensor(
                    gt8_t[0:T, :], tmp_t[0:T, BM, :], pl_t[0:T, BEST, t, :], ALU.is_gt
                )
                nc.vector.tensor_tensor(
                    pl_t[0:T, BEST, t, :], pl_t[0:T, BEST, t, :], tmp_t[0:T, BM, :], ALU.max
                )
                nc.vector.tensor_scalar(
                    tmp_t[0:T, CNT, :], tmp_t[0:T, CNT, :], -9.0, None, ALU.add, ALU.bypass
                )
                nc.vector.copy_predicated(
                    pl_t[0:T, FX, t, :], gt8_t[0:T, :], tmp_t[0:T, CNT, :]
                )
                dyb = dyv_t[0:T, dyi : dyi + 1].broadcast_to([T, JW])
                nc.vector.copy_predicated(pl_t[0:T, FY, t, :], gt8_t[0:T, :], dyb)

        for t, (i0, i1, _) in enumerate(OT):
            T = i1 - i0
            nc.sync.dma_start(fx_d[i0:i1, :], pl_t[0:T, FX, t, :])
            nc.sync.dma_start(fy_d[i0:i1, :], pl_t[0:T, FY, t, :])
            nc.sync.dma_start(cf_d[i0:i1, :], pl_t[0:T, CF, t, :])

    nc.compile()
    return nc


_NC = None


def get_program():
    global _NC
    if _NC is None:
        _NC = build_program()
    return _NC


def make_core_inputs(p1b, p2b, j0, bands, dyv):
    p1p = np.pad(p1b, ((10, 10), (11, 11)))
    p1s = p1p[:, j0 : j0 + XW].copy()
    q1p = np.pad((2.0 * p1b - 1.0).astype(np.float32), ((10, 10), (11, 11)))
    q1s = q1p[:, j0 : j0 + XW].copy()
    q1s[:, 0] = 0.0
    p2p = np.pad(p2b, ((19, 19), (20, 20)))
    p2s = p2p[:, j0 : j0 + P2W].copy()
    return {
        "p1s": np.ascontiguousarray(p1s, np.float32),
        "q1s": np.ascontiguousarray(q1s, np.float32),
        "p2s": np.ascontiguousarray(p2s, np.float32),
        "bands": bands,
        "dyv": dyv,
    }


def kernel(p1, p2):
    p1 = np.asarray(p1, np.float32)
    p2 = np.asarray(p2, np.float32)
    B = p1.shape[0]
    nc = get_program()
    bands = make_bands()
    dyv = np.tile(np.arange(-9, 10, dtype=np.float32), (128, 1))

    in_maps = []
    cores = []
    for b in range(B):
        for wh in range(2):
            j0 = wh * JW
            in_maps.append(make_core_inputs(p1[b, 0], p2[b, 0], j0, bands, dyv))
            cores.append((b, wh))

    res = run_bass_kernel_spmd(nc, in_maps, list(range(len(in_maps))))

    flow = np.zeros((B, 2, H, W), np.float32)
    conf = np.zeros((B, 1, H, W), np.float32)
    for i, (b, wh) in enumerate(cores):
        j0 = wh * JW
        r = res.results[i]
        flow[b, 0, :, j0 : j0 + JW] = r["fx"]
        flow[b, 1, :, j0 : j0 + JW] = r["fy"]
        conf[b, 0, :, j0 : j0 + JW] = r["cf"]
    return flow, conf
